# revision 42
# baseline (speedup 1.0000x reference)
"""DeepSeekMoE forward on 8 Trainium2 NeuronCores (Bass/Tile).

Strategy (expert parallelism per sharding hint):
  - 32 experts sharded 4-per-core; shared expert sharded over F (256/core);
    router replicated (gate weights permuted per-core so each core's own
    4 experts are always logit columns 0..3 -- keeps the SPMD program
    identical across cores).
  - Per core, fully on-device: router (logits -> top4 mask -> softmax
    weights cw), per-expert token compaction ranks (prefix-sum matmul with
    a strict-upper-triangular constant), token gather AND transpose in one
    matmul (xgT[D,C] = x[T,D].T-contraction with one-hot GT[T,C]),
    swiglu in capacity-C=256 gathered space, weighted scatter-add back via
    one-hot matmul fused with the shared-expert output into the same PSUM
    accumulation.
  - Host: shards/transposes weights (layout prep), sums the 8 per-core
    partial outputs (the all-reduce "combine" of expert partials).

The router runs in exact fp32 (top-k selection, counts and load_balance
match the reference bit-for-bit); the expert/shared compute path runs in
bf16 (fp32 PSUM accumulation), which halves weight DMA (the dominant
traffic: 48 MiB/core) and runs the PE at full 2.4 GHz rate. Capacity
C=256 per expert per core (actual max load for this problem size is ~146
of mean 128; overflow probability ~1e-8 for any same-shape input).

Measured on 8 axon TRN2 cores: 418 us HW exec, out rel-l2 4.3e-3 vs the
fp32 CPU reference, counts/load_balance exact.
"""

import numpy as np

import concourse.bass as bass
import concourse.bacc as bacc
import concourse.mybir as mybir
import concourse.tile as tile
from concourse.bass_utils import run_bass_kernel_spmd

F32 = mybir.dt.float32
F32R = mybir.dt.float32r
BF16 = mybir.dt.bfloat16
AL = mybir.AluOpType
AF = mybir.ActivationFunctionType
AX = mybir.AxisListType

P = 128
B, S, D, F, E = 2, 512, 1024, 2048, 32
T = B * S            # 1024 tokens
TOPK = 4
NCORES = 8
EPC = E // NCORES    # 4 experts per core
FS = F // NCORES     # 256 shared-expert hidden slice per core
C = 256              # per-expert token capacity (>= max load, mult of 128, >=256 for f32r)
NT = T // P          # 8 token tiles
ND = D // P          # 8 contraction chunks over D
NF = F // P          # 16 F tiles
NC_CHUNK = C // P    # 2 capacity chunks
FG = 512             # F-group width for wg/wu weight streaming (moving N)
NFG = F // FG        # 4

_NC_CACHE = None


def _mm(nc, out, lhsT, rhs, start, stop):
    nc.tensor.matmul(out, lhsT, rhs, start=start, stop=stop)


def build_nc():
    nc = bacc.Bacc("TRN2", target_bir_lowering=False, debug=False)

    # ---- I/O declarations (same names/shapes on every core) ----
    x_d = nc.dram_tensor("x", [T, D], BF16, kind="ExternalInput").ap()
    xT_d = nc.dram_tensor("xT", [D, T], F32, kind="ExternalInput").ap()
    xTb_d = nc.dram_tensor("xTb", [D, T], BF16, kind="ExternalInput").ap()
    gateT_d = nc.dram_tensor("gateT", [D, E], F32, kind="ExternalInput").ap()
    bias_d = nc.dram_tensor("bias_row", [P, E], F32, kind="ExternalInput").ap()
    wgT_d = nc.dram_tensor("wgT", [EPC, D, F], BF16, kind="ExternalInput").ap()
    wuT_d = nc.dram_tensor("wuT", [EPC, D, F], BF16, kind="ExternalInput").ap()
    wdT_d = nc.dram_tensor("wdT", [EPC, F, D], BF16, kind="ExternalInput").ap()
    wsgT_d = nc.dram_tensor("wsgT", [D, FS], BF16, kind="ExternalInput").ap()
    wsuT_d = nc.dram_tensor("wsuT", [D, FS], BF16, kind="ExternalInput").ap()
    wsdT_d = nc.dram_tensor("wsdT", [FS, D], BF16, kind="ExternalInput").ap()
    triu_d = nc.dram_tensor("triu", [P, P], F32, kind="ExternalInput").ap()
    ident_d = nc.dram_tensor("ident", [P, P], F32, kind="ExternalInput").ap()
    identb_d = nc.dram_tensor("identb", [P, P], BF16, kind="ExternalInput").ap()
    allones_d = nc.dram_tensor("allones", [P, P], F32, kind="ExternalInput").ap()
    iotaC_d = nc.dram_tensor("iotaC", [P, C], F32, kind="ExternalInput").ap()
    ones_d = nc.dram_tensor("ones", [P, 1], F32, kind="ExternalInput").ap()

    out_d = nc.dram_tensor("out_partial", [T, D], F32, kind="ExternalOutput").ap()
    counts_d = nc.dram_tensor("counts", [1, E], F32, kind="ExternalOutput").ap()
    lb_d = nc.dram_tensor("lb", [1, 1], F32, kind="ExternalOutput").ap()

    with tile.TileContext(nc) as tc:
        with (
            tc.tile_pool(name="persist", bufs=1) as pp,
            tc.tile_pool(name="rt", bufs=2) as rt,
            tc.tile_pool(name="psA", bufs=3, space="PSUM") as psA,
            tc.tile_pool(name="psB", bufs=4, space="PSUM") as psB,
            tc.tile_pool(name="psC", bufs=1, space="PSUM") as psC,
        ):
            # ---------- resident tensors ----------
            x_sb = pp.tile([P, NT, D], BF16, tag="x_sb")
            acc = pp.tile([P, NT, D], F32, tag="acc")
            bias_sb = pp.tile([P, E], F32, tag="bias")
            triu_sb = pp.tile([P, P], F32, tag="triu")
            ident_sb = pp.tile([P, P], F32, tag="ident")
            identb_sb = pp.tile([P, P], BF16, tag="identb")
            allones_sb = pp.tile([P, P], F32, tag="allones")
            iotaC_sb = pp.tile([P, C], F32, tag="iotaC")
            ones_sb = pp.tile([P, 1], F32, tag="ones")
            mask_sb = pp.tile([P, NT, E], F32, tag="mask")
            rank4_sb = pp.tile([P, NT, EPC], F32, tag="rank4")
            cw4_sb = pp.tile([P, NT, EPC], F32, tag="cw4")

            nc.sync.dma_start(out=x_sb, in_=x_d.rearrange("(n p) d -> p n d", p=P))
            nc.sync.dma_start(out=bias_sb, in_=bias_d)
            nc.sync.dma_start(out=triu_sb, in_=triu_d)
            nc.sync.dma_start(out=ident_sb, in_=ident_d)
            nc.sync.dma_start(out=identb_sb, in_=identb_d)
            nc.sync.dma_start(out=allones_sb, in_=allones_d)
            nc.sync.dma_start(out=iotaC_sb, in_=iotaC_d)
            nc.sync.dma_start(out=ones_sb, in_=ones_d)

            # ---------- early-phase tensors (released before expert loop) ----------
            early = tc.alloc_tile_pool(name="early", bufs=1)
            xT_sb = early.tile([P, ND, T], F32, tag="xT_sb")
            xTb_sb = early.tile([P, ND, T], BF16, tag="xTb_sb")
            gateT_sb = early.tile([P, ND, E], F32, tag="gateT")
            nc.sync.dma_start(out=xT_sb, in_=xT_d.rearrange("(n p) t -> p n t", p=P))
            nc.sync.dma_start(out=xTb_sb, in_=xTb_d.rearrange("(n p) t -> p n t", p=P))
            nc.sync.dma_start(out=gateT_sb, in_=gateT_d.rearrange("(n p) e -> p n e", p=P))

            # ---------- router ----------
            counts_ps = psC.tile([1, E], F32, tag="C")

            for tt in range(NT):
                lg_ps = psA.tile([P, E], F32, tag="A")
                for dc in range(ND):
                    nc.tensor.matmul(
                        lg_ps,
                        xT_sb[:, dc, tt * P:(tt + 1) * P],
                        gateT_sb[:, dc, :],
                        start=(dc == 0),
                        stop=(dc == ND - 1),
                    )
                logits = rt.tile([P, E], F32, tag="logits")
                nc.vector.tensor_tensor(logits, lg_ps, bias_sb, op=AL.add)
                max8 = rt.tile([P, 8], F32, tag="max8")
                nc.vector.max(out=max8, in_=logits)
                negm = rt.tile([P, 1], F32, tag="negm")
                nc.vector.tensor_scalar_mul(negm, max8[:, 0:1], -1.0)
                msk = mask_sb[:, tt, :]
                nc.vector.tensor_scalar(
                    msk, logits, max8[:, 3:4], None, op0=AL.is_ge
                )
                expv = rt.tile([P, E], F32, tag="expv")
                nc.scalar.activation(expv, logits, AF.Exp, bias=negm)
                mexp = rt.tile([P, E], F32, tag="mexp")
                nc.vector.tensor_mul(mexp, expv, msk)
                zs = rt.tile([P, 1], F32, tag="zs")
                nc.vector.reduce_sum(zs, mexp, axis=AX.X)
                rz = rt.tile([P, 1], F32, tag="rz")
                nc.vector.reciprocal(rz, zs)
                nc.vector.tensor_scalar(
                    cw4_sb[:, tt, :], mexp[:, 0:EPC], rz, None, op0=AL.mult
                )

                # counts (accumulated over tiles)
                nc.tensor.matmul(
                    counts_ps, ones_sb, msk, start=(tt == 0), stop=(tt == NT - 1)
                )

                # exclusive prefix ranks for this core's 4 experts (cols 0..3):
                # sum of full earlier tiles (all-ones lhsT) + strict within-tile
                # prefix (triu lhsT), all in one PSUM accumulation group.
                pre_ps = psA.tile([P, EPC], F32, tag="A")
                for i in range(tt):
                    nc.tensor.matmul(
                        pre_ps, allones_sb, mask_sb[:, i, 0:EPC],
                        start=(i == 0), stop=False,
                    )
                nc.tensor.matmul(
                    pre_ps, triu_sb, msk[:, 0:EPC], start=(tt == 0), stop=True
                )
                r4 = rt.tile([P, EPC], F32, tag="r4")
                nc.vector.tensor_scalar_add(r4, pre_ps, 1.0)
                nc.vector.tensor_mul(r4, r4, msk[:, 0:EPC])
                nc.vector.tensor_scalar_add(rank4_sb[:, tt, :], r4, -1.0)

            # ---------- counts stats ----------
            counts_sb = pp.tile([1, E], F32, tag="counts_sb")
            nc.vector.tensor_copy(counts_sb, counts_ps)
            nc.sync.dma_start(out=counts_d, in_=counts_sb)
            mean = rt.tile([1, 1], F32, tag="mean")
            nc.vector.reduce_sum(mean, counts_sb, axis=AX.X)
            nc.vector.tensor_scalar_mul(mean, mean, 1.0 / E)
            dev = rt.tile([1, E], F32, tag="dev")
            nc.vector.tensor_scalar(dev, counts_sb, mean, None, op0=AL.subtract)
            nc.vector.tensor_mul(dev, dev, dev)
            var = rt.tile([1, 1], F32, tag="var")
            nc.vector.reduce_sum(var, dev, axis=AX.X)
            nc.vector.tensor_scalar_mul(var, var, 1.0 / E)
            std = rt.tile([1, 1], F32, tag="std")
            nc.scalar.activation(std, var, AF.Sqrt)
            meps = rt.tile([1, 1], F32, tag="meps")
            nc.vector.tensor_scalar_add(meps, mean, 1e-6)
            rme = rt.tile([1, 1], F32, tag="rme")
            nc.vector.reciprocal(rme, meps)
            lb = rt.tile([1, 1], F32, tag="lb")
            nc.vector.tensor_mul(lb, std, rme)
            nc.sync.dma_start(out=lb_d, in_=lb)

            # ---------- shared expert (F-slice), writes acc ----------
            with tc.tile_pool(name="shared", bufs=1) as sh:
                wsgT_sb = sh.tile([P, ND, FS], BF16, tag="wsgT")
                wsuT_sb = sh.tile([P, ND, FS], BF16, tag="wsuT")
                wsdT_sb = sh.tile([P, FS // P, D], BF16, tag="wsdT")
                hsT = sh.tile([P, FS // P, T], BF16, tag="hsT")
                nc.sync.dma_start(
                    out=wsgT_sb, in_=wsgT_d.rearrange("(n p) f -> p n f", p=P)
                )
                nc.sync.dma_start(
                    out=wsuT_sb, in_=wsuT_d.rearrange("(n p) f -> p n f", p=P)
                )
                nc.sync.dma_start(
                    out=wsdT_sb, in_=wsdT_d.rearrange("(n p) d -> p n d", p=P)
                )
                for fs in range(FS // P):
                    for th in range(2):
                        tsl = slice(th * 512, (th + 1) * 512)
                        psg = psB.tile([P, 512], F32, tag="B")
                        psu = psB.tile([P, 512], F32, tag="B")
                        for dc in range(ND):
                            _mm(nc, psg, wsgT_sb[:, dc, fs * P:(fs + 1) * P],
                                xTb_sb[:, dc, tsl], dc == 0, dc == ND - 1)
                        for dc in range(ND):
                            _mm(nc, psu, wsuT_sb[:, dc, fs * P:(fs + 1) * P],
                                xTb_sb[:, dc, tsl], dc == 0, dc == ND - 1)
                        sil = rt.tile([P, 512], F32, tag="sil")
                        nc.scalar.activation(sil, psg, AF.Sigmoid)
                        nc.vector.tensor_mul(sil, sil, psg)
                        nc.vector.tensor_mul(hsT[:, fs, tsl], sil, psu)
                for tt in range(NT):
                    for dh in range(2):
                        dsl = slice(dh * 512, (dh + 1) * 512)
                        pso = psB.tile([P, 512], F32, tag="B")
                        for fs in range(FS // P):
                            _mm(nc, pso, hsT[:, fs, tt * P:(tt + 1) * P],
                                wsdT_sb[:, fs, dsl], fs == 0, fs == FS // P - 1)
                        nc.vector.tensor_copy(acc[:, tt, dsl], pso)

            early.release()

            # ---------- routed experts ----------
            with tc.tile_pool(name="exp", bufs=1) as ep, \
                 tc.tile_pool(name="wstream", bufs=2) as ws, \
                 tc.tile_pool(name="wdstream", bufs=2) as wds:
                for e in range(EPC):
                    # one-hot gather matrix GT[t, s] = (rank[t] == s)
                    GT = ep.tile([P, NT, C], BF16, tag="GT")
                    for tt in range(NT):
                        nc.vector.tensor_scalar(
                            GT[:, tt, :], iotaC_sb, rank4_sb[:, tt, e:e + 1],
                            None, op0=AL.is_equal,
                        )
                    # weighted scatter matrix S_w[s, t] = GT[t, s] * cw[t],
                    # built as per-partition-scalar multiply then PE-transposed
                    sw = ep.tile([P, NC_CHUNK, T], BF16, tag="sw")
                    for tt in range(NT):
                        swT_t = rt.tile([P, C], BF16, tag="swT_t")
                        nc.vector.tensor_scalar(
                            swT_t, GT[:, tt, :], cw4_sb[:, tt, e:e + 1],
                            None, op0=AL.mult,
                        )
                        for cc in range(NC_CHUNK):
                            tr_ps = psA.tile([P, P], BF16, tag="A")
                            nc.tensor.transpose(
                                tr_ps, swT_t[:, cc * P:(cc + 1) * P], identb_sb
                            )
                            nc.vector.tensor_copy(
                                sw[:, cc, tt * P:(tt + 1) * P], tr_ps
                            )

                    # gathered+transposed tokens xgT[D, C]
                    xgT = ep.tile([P, ND, C], BF16, tag="xgT")
                    for dc in range(ND):
                        g_ps = psA.tile([P, C], F32, tag="A")
                        for tt in range(NT):
                            _mm(nc, g_ps, x_sb[:, tt, dc * P:(dc + 1) * P],
                                GT[:, tt, :], tt == 0, tt == NT - 1)
                        nc.vector.tensor_copy(xgT[:, dc, :], g_ps)

                    # swiglu hidden + down-proj, y accumulated in 4 PSUM banks
                    y_ps = [
                        [psB.tile([P, 512], F32, tag="B", name=f"y_ps_{cc}_{dh}")
                         for dh in range(2)]
                        for cc in range(NC_CHUNK)
                    ]
                    # h-phase with xgT as the stationary operand and the
                    # weights moving at N=FG per matmul: halves the per-row
                    # LDWEIGHTS tax vs weights-stationary at N=C. Produces h
                    # in [C, F] orientation; cheap bf16 PE transposes recover
                    # hT chunks for the down-projection.
                    hT_e = ep.tile([P, NF, C], BF16, tag="hTe")
                    for fg in range(NFG):
                        wg_t = ws.tile([P, ND, FG], BF16, tag="wg")
                        wu_t = ws.tile([P, ND, FG], BF16, tag="wu")
                        fsl = slice(fg * FG, (fg + 1) * FG)
                        nc.sync.dma_start(
                            out=wg_t,
                            in_=wgT_d[e, :, fsl].rearrange("(n p) f -> p n f", p=P),
                        )
                        nc.sync.dma_start(
                            out=wu_t,
                            in_=wuT_d[e, :, fsl].rearrange("(n p) f -> p n f", p=P),
                        )
                        for cc in range(NC_CHUNK):
                            psg = psA.tile([P, FG], F32, tag="A")
                            psu = psA.tile([P, FG], F32, tag="A")
                            for dc in range(ND):
                                _mm(nc, psg, xgT[:, dc, cc * P:(cc + 1) * P],
                                    wg_t[:, dc, :], dc == 0, dc == ND - 1)
                            for dc in range(ND):
                                _mm(nc, psu, xgT[:, dc, cc * P:(cc + 1) * P],
                                    wu_t[:, dc, :], dc == 0, dc == ND - 1)
                            sil = rt.tile([P, FG], F32, tag="silh")
                            nc.scalar.activation(sil, psg, AF.Sigmoid)
                            nc.vector.tensor_mul(sil, sil, psg)
                            hch = rt.tile([P, FG], BF16, tag="hch")
                            nc.vector.tensor_mul(hch, sil, psu)
                            for fi in range(FG // P):
                                ft = fg * (FG // P) + fi
                                htr = psA.tile([P, P], BF16, tag="A")
                                nc.tensor.transpose(
                                    htr, hch[:, fi * P:(fi + 1) * P], identb_sb
                                )
                                nc.vector.tensor_copy(
                                    hT_e[:, ft, cc * P:(cc + 1) * P], htr
                                )
                        for fi in range(FG // P):
                            ft = fg * (FG // P) + fi
                            wd_t = wds.tile([P, D], BF16, tag="wd")
                            nc.sync.dma_start(
                                out=wd_t, in_=wdT_d[e, ft * P:(ft + 1) * P, :]
                            )
                            for cc in range(NC_CHUNK):
                                for dh in range(2):
                                    _mm(nc, y_ps[cc][dh],
                                        hT_e[:, ft, cc * P:(cc + 1) * P],
                                        wd_t[:, dh * 512:(dh + 1) * 512],
                                        ft == 0, ft == NF - 1)
                    y_sb = ep.tile([P, NC_CHUNK, D], BF16, tag="y_sb")
                    for cc in range(NC_CHUNK):
                        for dh in range(2):
                            nc.vector.tensor_copy(
                                y_sb[:, cc, dh * 512:(dh + 1) * 512], y_ps[cc][dh]
                            )

                    for tt in range(NT):
                        for dh in range(2):
                            dsl = slice(dh * 512, (dh + 1) * 512)
                            pso = psB.tile([P, 512], F32, tag="B")
                            for cc in range(NC_CHUNK):
                                _mm(nc, pso, sw[:, cc, tt * P:(tt + 1) * P],
                                    y_sb[:, cc, dsl], cc == 0, cc == NC_CHUNK - 1)
                            nc.vector.tensor_add(acc[:, tt, dsl], acc[:, tt, dsl], pso)

            nc.sync.dma_start(out=out_d.rearrange("(n p) d -> p n d", p=P), in_=acc)

    nc.compile()
    return nc


def make_in_maps(inputs):
    x = np.ascontiguousarray(inputs["x"].reshape(T, D).astype(np.float32))
    gate_w = np.asarray(inputs["gate_w"], np.float32)
    expert_bias = np.asarray(inputs["expert_bias"], np.float32)
    wg = np.asarray(inputs["wg"], np.float32)
    wu = np.asarray(inputs["wu"], np.float32)
    wd = np.asarray(inputs["wd"], np.float32)
    ws_g = np.asarray(inputs["ws_g"], np.float32)
    ws_u = np.asarray(inputs["ws_u"], np.float32)
    ws_d = np.asarray(inputs["ws_d"], np.float32)

    xT = np.ascontiguousarray(x.T)
    triu = np.triu(np.ones((P, P), np.float32), k=1)
    ident = np.eye(P, dtype=np.float32)
    allones = np.ones((P, P), np.float32)
    iotaC = np.broadcast_to(np.arange(C, dtype=np.float32), (P, C)).copy()
    ones = np.ones((P, 1), np.float32)

    import ml_dtypes
    bf16 = ml_dtypes.bfloat16
    x_b = x.astype(bf16)
    xTb = np.ascontiguousarray(x.T).astype(bf16)
    in_maps = []
    for c in range(NCORES):
        own = list(range(c * EPC, (c + 1) * EPC))
        perm = own + [i for i in range(E) if i not in own]
        fsl = slice(c * FS, (c + 1) * FS)
        in_maps.append({
            "x": x_b,
            "xT": xT,
            "xTb": xTb,
            "gateT": np.ascontiguousarray(gate_w[perm].T),
            "bias_row": np.broadcast_to(expert_bias[perm], (P, E)).copy(),
            "wgT": np.ascontiguousarray(wg[own].transpose(0, 2, 1)).astype(bf16),
            "wuT": np.ascontiguousarray(wu[own].transpose(0, 2, 1)).astype(bf16),
            "wdT": np.ascontiguousarray(wd[own].transpose(0, 2, 1)).astype(bf16),
            "wsgT": np.ascontiguousarray(ws_g[fsl, :].T).astype(bf16),
            "wsuT": np.ascontiguousarray(ws_u[fsl, :].T).astype(bf16),
            "wsdT": np.ascontiguousarray(ws_d[:, fsl].T).astype(bf16),
            "triu": triu,
            "ident": ident,
            "identb": ident.astype(bf16),
            "allones": allones,
            "iotaC": iotaC,
            "ones": ones,
        })
    return in_maps


def kernel(x, gate_w, expert_bias, wg, wu, wd, ws_g, ws_u, ws_d, **run_kwargs):
    global _NC_CACHE
    inputs = dict(x=x, gate_w=gate_w, expert_bias=expert_bias, wg=wg, wu=wu,
                  wd=wd, ws_g=ws_g, ws_u=ws_u, ws_d=ws_d)
    in_maps = make_in_maps(inputs)
    if _NC_CACHE is None:
        _NC_CACHE = build_nc()
    nc = _NC_CACHE
    res = run_bass_kernel_spmd(nc, in_maps, list(range(NCORES)), **run_kwargs)
    parts = res.results
    out = np.zeros((T, D), np.float64)
    for c in range(NCORES):
        out += parts[c]["out_partial"].astype(np.float64)
    out = out.astype(np.float32).reshape(B, S, D)
    counts = parts[0]["counts"].reshape(E).astype(np.float32)
    lb = np.float32(parts[0]["lb"].reshape(-1)[0])
    if run_kwargs:
        return (out, counts, lb), res
    return out, counts, lb


# revision 44
# speedup vs baseline: 1.0732x; 1.0732x over previous
"""DeepSeekMoE forward on 8 Trainium2 NeuronCores (Bass/Tile).

Strategy (expert parallelism per sharding hint):
  - 32 experts sharded 4-per-core; shared expert sharded over F (256/core);
    router replicated (gate weights permuted per-core so each core's own
    4 experts are always logit columns 0..3 -- keeps the SPMD program
    identical across cores).
  - Per core, fully on-device: router (logits -> top4 mask -> softmax
    weights cw), per-expert token compaction ranks (prefix-sum matmul with
    a strict-upper-triangular constant), token gather AND transpose in one
    matmul (xgT[D,C] = x[T,D].T-contraction with one-hot GT[T,C]),
    swiglu in capacity-C=256 gathered space, weighted scatter-add back via
    one-hot matmul fused with the shared-expert output into the same PSUM
    accumulation.
  - Host: shards/transposes weights (layout prep), sums the 8 per-core
    partial outputs (the all-reduce "combine" of expert partials).

The router runs in exact fp32 (top-k selection, counts and load_balance
match the reference bit-for-bit); the expert/shared compute path runs in
bf16 (fp32 PSUM accumulation), which halves weight DMA (the dominant
traffic, 48 MiB/core) and runs the PE at full rate. Capacity C=256 per
expert per core (actual max load ~146 of mean 128; overflow probability
~1e-8 for any same-shape input).

Measured on 8 axon TRN2 cores: ~420 us HW exec, out rel-l2 4.3e-3 vs the
fp32 CPU reference, counts/load_balance exact.
"""

import math
import numpy as np

import concourse.bass as bass
import concourse.bacc as bacc
import concourse.mybir as mybir
import concourse.tile as tile
from concourse.bass_utils import run_bass_kernel_spmd

F32 = mybir.dt.float32
F32R = mybir.dt.float32r
BF16 = mybir.dt.bfloat16
AL = mybir.AluOpType
AF = mybir.ActivationFunctionType
AX = mybir.AxisListType

P = 128
B, S, D, F, E = 2, 512, 1024, 2048, 32
T = B * S            # 1024 tokens
TOPK = 4
NCORES = 8
EPC = E // NCORES    # 4 experts per core
FS = F // NCORES     # 256 shared-expert hidden slice per core
C = 256              # per-expert token capacity (>= max load, mult of 128, >=256 for f32r)
NT = T // P          # 8 token tiles
ND = D // P          # 8 contraction chunks over D
NF = F // P          # 16 F tiles
NC_CHUNK = C // P    # 2 capacity chunks
FG = 256             # F-group width for wg/wu weight streaming
NFG = F // FG        # 8

_NC_CACHE = None


def _mm(nc, out, lhsT, rhs, start, stop):
    """float32r matmul (full-rate 4-byte path; operands are f32r tiles)."""
    nc.tensor.matmul(out, lhsT, rhs, start=start, stop=stop)


def round_f32r(a):
    """Round fp32 array to the fp32r grid (11 mantissa bits, RNE)."""
    u = np.ascontiguousarray(a, np.float32).view(np.uint32)
    r = (u + np.uint32(0x800) + ((u >> np.uint32(12)) & np.uint32(1))) & np.uint32(
        0xFFFFF000
    )
    return r.view(np.float32)


def build_nc():
    nc = bacc.Bacc("TRN2", target_bir_lowering=False, debug=False)

    # ---- I/O declarations (same names/shapes on every core) ----
    x_d = nc.dram_tensor("x", [T, D], BF16, kind="ExternalInput").ap()
    xT_d = nc.dram_tensor("xT", [D, T], F32, kind="ExternalInput").ap()
    xTb_d = nc.dram_tensor("xTb", [D, T], BF16, kind="ExternalInput").ap()
    gateT_d = nc.dram_tensor("gateT", [D, E], F32, kind="ExternalInput").ap()
    bias_d = nc.dram_tensor("bias_row", [P, E], F32, kind="ExternalInput").ap()
    wgT_d = nc.dram_tensor("wgT", [EPC, D, F], BF16, kind="ExternalInput").ap()
    wuT_d = nc.dram_tensor("wuT", [EPC, D, F], BF16, kind="ExternalInput").ap()
    wdT_d = nc.dram_tensor("wdT", [EPC, F, D], BF16, kind="ExternalInput").ap()
    wsgT_d = nc.dram_tensor("wsgT", [D, FS], BF16, kind="ExternalInput").ap()
    wsuT_d = nc.dram_tensor("wsuT", [D, FS], BF16, kind="ExternalInput").ap()
    wsdT_d = nc.dram_tensor("wsdT", [FS, D], BF16, kind="ExternalInput").ap()
    triu_d = nc.dram_tensor("triu", [P, P], F32, kind="ExternalInput").ap()
    ident_d = nc.dram_tensor("ident", [P, P], F32, kind="ExternalInput").ap()
    identb_d = nc.dram_tensor("identb", [P, P], BF16, kind="ExternalInput").ap()
    allones_d = nc.dram_tensor("allones", [P, P], F32, kind="ExternalInput").ap()
    iotaC_d = nc.dram_tensor("iotaC", [P, C], F32, kind="ExternalInput").ap()
    ones_d = nc.dram_tensor("ones", [P, 1], F32, kind="ExternalInput").ap()

    out_d = nc.dram_tensor("out_partial", [T, D], F32, kind="ExternalOutput").ap()
    counts_d = nc.dram_tensor("counts", [1, E], F32, kind="ExternalOutput").ap()
    lb_d = nc.dram_tensor("lb", [1, 1], F32, kind="ExternalOutput").ap()

    with tile.TileContext(nc) as tc:
        with (
            tc.tile_pool(name="persist", bufs=1) as pp,
            tc.tile_pool(name="rt", bufs=2) as rt,
            tc.tile_pool(name="psA", bufs=3, space="PSUM") as psA,
            tc.tile_pool(name="psB", bufs=4, space="PSUM") as psB,
            tc.tile_pool(name="psC", bufs=1, space="PSUM") as psC,
        ):
            # ---------- resident tensors ----------
            x_sb = pp.tile([P, NT, D], BF16, tag="x_sb")
            acc = pp.tile([P, NT, D], F32, tag="acc")
            bias_sb = pp.tile([P, E], F32, tag="bias")
            triu_sb = pp.tile([P, P], F32, tag="triu")
            ident_sb = pp.tile([P, P], F32, tag="ident")
            identb_sb = pp.tile([P, P], BF16, tag="identb")
            allones_sb = pp.tile([P, P], F32, tag="allones")
            iotaC_sb = pp.tile([P, C], F32, tag="iotaC")
            ones_sb = pp.tile([P, 1], F32, tag="ones")
            mask_sb = pp.tile([P, NT, E], F32, tag="mask")
            rank4_sb = pp.tile([P, NT, EPC], F32, tag="rank4")
            cw4_sb = pp.tile([P, NT, EPC], F32, tag="cw4")

            nc.sync.dma_start(out=x_sb, in_=x_d.rearrange("(n p) d -> p n d", p=P))
            nc.sync.dma_start(out=bias_sb, in_=bias_d)
            nc.sync.dma_start(out=triu_sb, in_=triu_d)
            nc.sync.dma_start(out=ident_sb, in_=ident_d)
            nc.sync.dma_start(out=identb_sb, in_=identb_d)
            nc.sync.dma_start(out=allones_sb, in_=allones_d)
            nc.sync.dma_start(out=iotaC_sb, in_=iotaC_d)
            nc.sync.dma_start(out=ones_sb, in_=ones_d)

            # ---------- early-phase tensors (released before expert loop) ----------
            early = tc.alloc_tile_pool(name="early", bufs=1)
            xT_sb = early.tile([P, ND, T], F32, tag="xT_sb")
            xTb_sb = early.tile([P, ND, T], BF16, tag="xTb_sb")
            gateT_sb = early.tile([P, ND, E], F32, tag="gateT")
            nc.sync.dma_start(out=xT_sb, in_=xT_d.rearrange("(n p) t -> p n t", p=P))
            nc.sync.dma_start(out=xTb_sb, in_=xTb_d.rearrange("(n p) t -> p n t", p=P))
            nc.sync.dma_start(out=gateT_sb, in_=gateT_d.rearrange("(n p) e -> p n e", p=P))

            # ---------- router ----------
            counts_ps = psC.tile([1, E], F32, tag="C")

            for tt in range(NT):
                lg_ps = psA.tile([P, E], F32, tag="A")
                for dc in range(ND):
                    nc.tensor.matmul(
                        lg_ps,
                        xT_sb[:, dc, tt * P:(tt + 1) * P],
                        gateT_sb[:, dc, :],
                        start=(dc == 0),
                        stop=(dc == ND - 1),
                    )
                logits = rt.tile([P, E], F32, tag="logits")
                nc.vector.tensor_tensor(logits, lg_ps, bias_sb, op=AL.add)
                max8 = rt.tile([P, 8], F32, tag="max8")
                nc.vector.max(out=max8, in_=logits)
                negm = rt.tile([P, 1], F32, tag="negm")
                nc.vector.tensor_scalar_mul(negm, max8[:, 0:1], -1.0)
                msk = mask_sb[:, tt, :]
                nc.vector.tensor_scalar(
                    msk, logits, max8[:, 3:4], None, op0=AL.is_ge
                )
                expv = rt.tile([P, E], F32, tag="expv")
                nc.scalar.activation(expv, logits, AF.Exp, bias=negm)
                mexp = rt.tile([P, E], F32, tag="mexp")
                nc.vector.tensor_mul(mexp, expv, msk)
                zs = rt.tile([P, 1], F32, tag="zs")
                nc.vector.reduce_sum(zs, mexp, axis=AX.X)
                rz = rt.tile([P, 1], F32, tag="rz")
                nc.vector.reciprocal(rz, zs)
                nc.vector.tensor_scalar(
                    cw4_sb[:, tt, :], mexp[:, 0:EPC], rz, None, op0=AL.mult
                )

                # counts (accumulated over tiles)
                nc.tensor.matmul(
                    counts_ps, ones_sb, msk, start=(tt == 0), stop=(tt == NT - 1)
                )

                # exclusive prefix ranks for this core's 4 experts (cols 0..3):
                # sum of full earlier tiles (all-ones lhsT) + strict within-tile
                # prefix (triu lhsT), all in one PSUM accumulation group.
                pre_ps = psA.tile([P, EPC], F32, tag="A")
                for i in range(tt):
                    nc.tensor.matmul(
                        pre_ps, allones_sb, mask_sb[:, i, 0:EPC],
                        start=(i == 0), stop=False,
                    )
                nc.tensor.matmul(
                    pre_ps, triu_sb, msk[:, 0:EPC], start=(tt == 0), stop=True
                )
                r4 = rt.tile([P, EPC], F32, tag="r4")
                nc.vector.tensor_scalar_add(r4, pre_ps, 1.0)
                nc.vector.tensor_mul(r4, r4, msk[:, 0:EPC])
                nc.vector.tensor_scalar_add(rank4_sb[:, tt, :], r4, -1.0)

            # ---------- counts stats ----------
            counts_sb = pp.tile([1, E], F32, tag="counts_sb")
            nc.vector.tensor_copy(counts_sb, counts_ps)
            nc.sync.dma_start(out=counts_d, in_=counts_sb)
            mean = rt.tile([1, 1], F32, tag="mean")
            nc.vector.reduce_sum(mean, counts_sb, axis=AX.X)
            nc.vector.tensor_scalar_mul(mean, mean, 1.0 / E)
            dev = rt.tile([1, E], F32, tag="dev")
            nc.vector.tensor_scalar(dev, counts_sb, mean, None, op0=AL.subtract)
            nc.vector.tensor_mul(dev, dev, dev)
            var = rt.tile([1, 1], F32, tag="var")
            nc.vector.reduce_sum(var, dev, axis=AX.X)
            nc.vector.tensor_scalar_mul(var, var, 1.0 / E)
            std = rt.tile([1, 1], F32, tag="std")
            nc.scalar.activation(std, var, AF.Sqrt)
            meps = rt.tile([1, 1], F32, tag="meps")
            nc.vector.tensor_scalar_add(meps, mean, 1e-6)
            rme = rt.tile([1, 1], F32, tag="rme")
            nc.vector.reciprocal(rme, meps)
            lb = rt.tile([1, 1], F32, tag="lb")
            nc.vector.tensor_mul(lb, std, rme)
            nc.sync.dma_start(out=lb_d, in_=lb)

            # ---------- shared expert (F-slice), writes acc ----------
            with tc.tile_pool(name="shared", bufs=1) as sh:
                wsgT_sb = sh.tile([P, ND, FS], BF16, tag="wsgT")
                wsuT_sb = sh.tile([P, ND, FS], BF16, tag="wsuT")
                wsdT_sb = sh.tile([P, FS // P, D], BF16, tag="wsdT")
                hsT = sh.tile([P, FS // P, T], BF16, tag="hsT")
                nc.sync.dma_start(
                    out=wsgT_sb, in_=wsgT_d.rearrange("(n p) f -> p n f", p=P)
                )
                nc.sync.dma_start(
                    out=wsuT_sb, in_=wsuT_d.rearrange("(n p) f -> p n f", p=P)
                )
                nc.sync.dma_start(
                    out=wsdT_sb, in_=wsdT_d.rearrange("(n p) d -> p n d", p=P)
                )
                for fs in range(FS // P):
                    for th in range(2):
                        tsl = slice(th * 512, (th + 1) * 512)
                        psg = psB.tile([P, 512], F32, tag="B")
                        psu = psB.tile([P, 512], F32, tag="B")
                        for dc in range(ND):
                            _mm(nc, psg, wsgT_sb[:, dc, fs * P:(fs + 1) * P],
                                xTb_sb[:, dc, tsl], dc == 0, dc == ND - 1)
                        for dc in range(ND):
                            _mm(nc, psu, wsuT_sb[:, dc, fs * P:(fs + 1) * P],
                                xTb_sb[:, dc, tsl], dc == 0, dc == ND - 1)
                        sil = rt.tile([P, 512], F32, tag="sil")
                        nc.scalar.activation(sil, psg, AF.Sigmoid)
                        nc.vector.tensor_mul(sil, sil, psg)
                        nc.vector.tensor_mul(hsT[:, fs, tsl], sil, psu)
                for tt in range(NT):
                    for dh in range(2):
                        dsl = slice(dh * 512, (dh + 1) * 512)
                        pso = psB.tile([P, 512], F32, tag="B")
                        for fs in range(FS // P):
                            _mm(nc, pso, hsT[:, fs, tt * P:(tt + 1) * P],
                                wsdT_sb[:, fs, dsl], fs == 0, fs == FS // P - 1)
                        nc.vector.tensor_copy(acc[:, tt, dsl], pso)

            early.release()

            # ---------- routed experts ----------
            with tc.tile_pool(name="exp", bufs=1) as ep, \
                 tc.tile_pool(name="wstream", bufs=2) as ws, \
                 tc.tile_pool(name="wdstream", bufs=2) as wds:
                for e in range(EPC):
                    # one-hot gather matrix GT[t, s] = (rank[t] == s)
                    GT = ep.tile([P, NT, C], BF16, tag="GT")
                    for tt in range(NT):
                        nc.vector.tensor_scalar(
                            GT[:, tt, :], iotaC_sb, rank4_sb[:, tt, e:e + 1],
                            None, op0=AL.is_equal,
                        )
                    # weighted scatter matrix S_w[s, t] = GT[t, s] * cw[t],
                    # built as per-partition-scalar multiply then PE-transposed
                    sw = ep.tile([P, NC_CHUNK, T], BF16, tag="sw")
                    for tt in range(NT):
                        swT_t = rt.tile([P, C], BF16, tag="swT_t")
                        nc.vector.tensor_scalar(
                            swT_t, GT[:, tt, :], cw4_sb[:, tt, e:e + 1],
                            None, op0=AL.mult,
                        )
                        for cc in range(NC_CHUNK):
                            tr_ps = psA.tile([P, P], BF16, tag="A")
                            nc.tensor.transpose(
                                tr_ps, swT_t[:, cc * P:(cc + 1) * P], identb_sb
                            )
                            nc.vector.tensor_copy(
                                sw[:, cc, tt * P:(tt + 1) * P], tr_ps
                            )

                    # gathered+transposed tokens xgT[D, C]
                    xgT = ep.tile([P, ND, C], BF16, tag="xgT")
                    for dc in range(ND):
                        g_ps = psA.tile([P, C], F32, tag="A")
                        for tt in range(NT):
                            _mm(nc, g_ps, x_sb[:, tt, dc * P:(dc + 1) * P],
                                GT[:, tt, :], tt == 0, tt == NT - 1)
                        nc.vector.tensor_copy(xgT[:, dc, :], g_ps)

                    # swiglu hidden + down-proj, y accumulated in 4 PSUM banks
                    y_ps = [
                        [psB.tile([P, 512], F32, tag="B", name=f"y_ps_{cc}_{dh}")
                         for dh in range(2)]
                        for cc in range(NC_CHUNK)
                    ]
                    for fg in range(NFG):
                        wg_t = ws.tile([P, ND, FG], BF16, tag="wg")
                        wu_t = ws.tile([P, ND, FG], BF16, tag="wu")
                        fsl = slice(fg * FG, (fg + 1) * FG)
                        nc.sync.dma_start(
                            out=wg_t,
                            in_=wgT_d[e, :, fsl].rearrange("(n p) f -> p n f", p=P),
                        )
                        nc.sync.dma_start(
                            out=wu_t,
                            in_=wuT_d[e, :, fsl].rearrange("(n p) f -> p n f", p=P),
                        )
                        for fi in range(FG // P):
                            ft = fg * (FG // P) + fi
                            psg = psA.tile([P, C], F32, tag="A")
                            psu = psA.tile([P, C], F32, tag="A")
                            for dc in range(ND):
                                _mm(nc, psg, wg_t[:, dc, fi * P:(fi + 1) * P],
                                    xgT[:, dc, :], dc == 0, dc == ND - 1)
                            for dc in range(ND):
                                _mm(nc, psu, wu_t[:, dc, fi * P:(fi + 1) * P],
                                    xgT[:, dc, :], dc == 0, dc == ND - 1)
                            sil = rt.tile([P, C], F32, tag="silh")
                            nc.scalar.activation(sil, psg, AF.Sigmoid)
                            nc.vector.tensor_mul(sil, sil, psg)
                            hT = ep.tile([P, C], BF16, tag="hT")
                            nc.vector.tensor_mul(hT, sil, psu)
                            wd_t = wds.tile([P, D], BF16, tag="wd")
                            nc.sync.dma_start(
                                out=wd_t, in_=wdT_d[e, ft * P:(ft + 1) * P, :]
                            )
                            for cc in range(NC_CHUNK):
                                for dh in range(2):
                                    _mm(nc, y_ps[cc][dh],
                                        hT[:, cc * P:(cc + 1) * P],
                                        wd_t[:, dh * 512:(dh + 1) * 512],
                                        ft == 0, ft == NF - 1)
                    y_sb = ep.tile([P, NC_CHUNK, D], BF16, tag="y_sb")
                    for cc in range(NC_CHUNK):
                        for dh in range(2):
                            nc.vector.tensor_copy(
                                y_sb[:, cc, dh * 512:(dh + 1) * 512], y_ps[cc][dh]
                            )

                    for tt in range(NT):
                        for dh in range(2):
                            dsl = slice(dh * 512, (dh + 1) * 512)
                            pso = psB.tile([P, 512], F32, tag="B")
                            for cc in range(NC_CHUNK):
                                _mm(nc, pso, sw[:, cc, tt * P:(tt + 1) * P],
                                    y_sb[:, cc, dsl], cc == 0, cc == NC_CHUNK - 1)
                            nc.vector.tensor_add(acc[:, tt, dsl], acc[:, tt, dsl], pso)

            nc.sync.dma_start(out=out_d.rearrange("(n p) d -> p n d", p=P), in_=acc)

    nc.compile()
    return nc


def make_in_maps(inputs):
    x = np.ascontiguousarray(inputs["x"].reshape(T, D).astype(np.float32))
    gate_w = np.asarray(inputs["gate_w"], np.float32)
    expert_bias = np.asarray(inputs["expert_bias"], np.float32)
    wg = np.asarray(inputs["wg"], np.float32)
    wu = np.asarray(inputs["wu"], np.float32)
    wd = np.asarray(inputs["wd"], np.float32)
    ws_g = np.asarray(inputs["ws_g"], np.float32)
    ws_u = np.asarray(inputs["ws_u"], np.float32)
    ws_d = np.asarray(inputs["ws_d"], np.float32)

    xT = np.ascontiguousarray(x.T)
    triu = np.triu(np.ones((P, P), np.float32), k=1)
    ident = np.eye(P, dtype=np.float32)
    allones = np.ones((P, P), np.float32)
    iotaC = np.broadcast_to(np.arange(C, dtype=np.float32), (P, C)).copy()
    ones = np.ones((P, 1), np.float32)

    import ml_dtypes
    bf16 = ml_dtypes.bfloat16
    x_b = x.astype(bf16)
    xTb = np.ascontiguousarray(x.T).astype(bf16)
    in_maps = []
    for c in range(NCORES):
        own = list(range(c * EPC, (c + 1) * EPC))
        perm = own + [i for i in range(E) if i not in own]
        fsl = slice(c * FS, (c + 1) * FS)
        in_maps.append({
            "x": x_b,
            "xT": xT,
            "xTb": xTb,
            "gateT": np.ascontiguousarray(gate_w[perm].T),
            "bias_row": np.broadcast_to(expert_bias[perm], (P, E)).copy(),
            "wgT": np.ascontiguousarray(wg[own].transpose(0, 2, 1)).astype(bf16),
            "wuT": np.ascontiguousarray(wu[own].transpose(0, 2, 1)).astype(bf16),
            "wdT": np.ascontiguousarray(wd[own].transpose(0, 2, 1)).astype(bf16),
            "wsgT": np.ascontiguousarray(ws_g[fsl, :].T).astype(bf16),
            "wsuT": np.ascontiguousarray(ws_u[fsl, :].T).astype(bf16),
            "wsdT": np.ascontiguousarray(ws_d[:, fsl].T).astype(bf16),
            "triu": triu,
            "ident": ident,
            "identb": ident.astype(bf16),
            "allones": allones,
            "iotaC": iotaC,
            "ones": ones,
        })
    return in_maps


def kernel(x, gate_w, expert_bias, wg, wu, wd, ws_g, ws_u, ws_d, **run_kwargs):
    global _NC_CACHE
    inputs = dict(x=x, gate_w=gate_w, expert_bias=expert_bias, wg=wg, wu=wu,
                  wd=wd, ws_g=ws_g, ws_u=ws_u, ws_d=ws_d)
    in_maps = make_in_maps(inputs)
    if _NC_CACHE is None:
        _NC_CACHE = build_nc()
    nc = _NC_CACHE
    res = run_bass_kernel_spmd(nc, in_maps, list(range(NCORES)), **run_kwargs)
    parts = res.results
    out = np.zeros((T, D), np.float64)
    for c in range(NCORES):
        out += parts[c]["out_partial"].astype(np.float64)
    out = out.astype(np.float32).reshape(B, S, D)
    counts = parts[0]["counts"].reshape(E).astype(np.float32)
    lb = np.float32(parts[0]["lb"].reshape(-1)[0])
    if run_kwargs:
        return (out, counts, lb), res
    return out, counts, lb


# revision 45
# speedup vs baseline: 1.1497x; 1.0713x over previous
"""DeepSeekMoE forward on 8 Trainium2 NeuronCores (Bass/Tile).

Strategy (expert parallelism per sharding hint):
  - 32 experts sharded 4-per-core; shared expert sharded over F (256/core);
    router replicated (gate weights permuted per-core so each core's own
    4 experts are always logit columns 0..3 -- keeps the SPMD program
    identical across cores).
  - Per core, fully on-device: router (logits -> top4 mask -> softmax
    weights cw), per-expert token compaction ranks (prefix-sum matmul with
    a strict-upper-triangular constant), token gather AND transpose in one
    matmul (xgT[D,C] = x[T,D].T-contraction with one-hot GT[T,C]),
    swiglu in capacity-C=256 gathered space, weighted scatter-add back via
    one-hot matmul fused with the shared-expert output into the same PSUM
    accumulation.
  - Host: shards/transposes weights (layout prep), sums the 8 per-core
    partial outputs (the all-reduce "combine" of expert partials).

The router runs in exact fp32 (top-k selection, counts and load_balance
match the reference bit-for-bit); the expert/shared compute path runs in
bf16 (fp32 PSUM accumulation), which halves weight DMA (the dominant
traffic, 48 MiB/core) and runs the PE at full rate. Capacity C=256 per
expert per core (actual max load ~146 of mean 128; overflow probability
~1e-8 for any same-shape input).

Measured on 8 axon TRN2 cores: ~420 us HW exec, out rel-l2 4.3e-3 vs the
fp32 CPU reference, counts/load_balance exact.
"""

import math
import numpy as np

import concourse.bass as bass
import concourse.bacc as bacc
import concourse.mybir as mybir
import concourse.tile as tile
from concourse.bass_utils import run_bass_kernel_spmd

F32 = mybir.dt.float32
F32R = mybir.dt.float32r
BF16 = mybir.dt.bfloat16
AL = mybir.AluOpType
AF = mybir.ActivationFunctionType
AX = mybir.AxisListType

P = 128
B, S, D, F, E = 2, 512, 1024, 2048, 32
T = B * S            # 1024 tokens
TOPK = 4
NCORES = 8
EPC = E // NCORES    # 4 experts per core
FS = F // NCORES     # 256 shared-expert hidden slice per core
C = 256              # per-expert token capacity (>= max load, mult of 128, >=256 for f32r)
NT = T // P          # 8 token tiles
ND = D // P          # 8 contraction chunks over D
NF = F // P          # 16 F tiles
NC_CHUNK = C // P    # 2 capacity chunks
FG = 256             # F-group width for wg/wu weight streaming
NFG = F // FG        # 8

_NC_CACHE = None


def _mm(nc, out, lhsT, rhs, start, stop):
    """float32r matmul (full-rate 4-byte path; operands are f32r tiles)."""
    nc.tensor.matmul(out, lhsT, rhs, start=start, stop=stop)


def round_f32r(a):
    """Round fp32 array to the fp32r grid (11 mantissa bits, RNE)."""
    u = np.ascontiguousarray(a, np.float32).view(np.uint32)
    r = (u + np.uint32(0x800) + ((u >> np.uint32(12)) & np.uint32(1))) & np.uint32(
        0xFFFFF000
    )
    return r.view(np.float32)


def build_nc():
    nc = bacc.Bacc("TRN2", target_bir_lowering=False, debug=False)

    # ---- I/O declarations (same names/shapes on every core) ----
    x_d = nc.dram_tensor("x", [T, D], BF16, kind="ExternalInput").ap()
    xT_d = nc.dram_tensor("xT", [D, T], F32, kind="ExternalInput").ap()
    xTb_d = nc.dram_tensor("xTb", [D, T], BF16, kind="ExternalInput").ap()
    gateT_d = nc.dram_tensor("gateT", [D, E], F32, kind="ExternalInput").ap()
    bias_d = nc.dram_tensor("bias_row", [P, E], F32, kind="ExternalInput").ap()
    wgT_d = nc.dram_tensor("wgT", [EPC, D, F], BF16, kind="ExternalInput").ap()
    wuT_d = nc.dram_tensor("wuT", [EPC, D, F], BF16, kind="ExternalInput").ap()
    wdT_d = nc.dram_tensor("wdT", [EPC, F, D], BF16, kind="ExternalInput").ap()
    wsgT_d = nc.dram_tensor("wsgT", [D, FS], BF16, kind="ExternalInput").ap()
    wsuT_d = nc.dram_tensor("wsuT", [D, FS], BF16, kind="ExternalInput").ap()
    wsdT_d = nc.dram_tensor("wsdT", [FS, D], BF16, kind="ExternalInput").ap()
    triu_d = nc.dram_tensor("triu", [P, P], F32, kind="ExternalInput").ap()
    ident_d = nc.dram_tensor("ident", [P, P], F32, kind="ExternalInput").ap()
    identb_d = nc.dram_tensor("identb", [P, P], BF16, kind="ExternalInput").ap()
    allones_d = nc.dram_tensor("allones", [P, P], F32, kind="ExternalInput").ap()
    iotaC_d = nc.dram_tensor("iotaC", [P, C], F32, kind="ExternalInput").ap()
    ones_d = nc.dram_tensor("ones", [P, 1], F32, kind="ExternalInput").ap()

    out_d = nc.dram_tensor("out_partial", [T, D], F32, kind="ExternalOutput").ap()
    counts_d = nc.dram_tensor("counts", [1, E], F32, kind="ExternalOutput").ap()
    lb_d = nc.dram_tensor("lb", [1, 1], F32, kind="ExternalOutput").ap()

    with tile.TileContext(nc) as tc:
        with (
            tc.tile_pool(name="persist", bufs=1) as pp,
            tc.tile_pool(name="rt", bufs=2) as rt,
            tc.tile_pool(name="psA", bufs=3, space="PSUM") as psA,
            tc.tile_pool(name="psB", bufs=4, space="PSUM") as psB,
            tc.tile_pool(name="psC", bufs=1, space="PSUM") as psC,
        ):
            # ---------- resident tensors ----------
            x_sb = pp.tile([P, NT, D], BF16, tag="x_sb")
            acc = pp.tile([P, NT, D], F32, tag="acc")
            bias_sb = pp.tile([P, E], F32, tag="bias")
            triu_sb = pp.tile([P, P], F32, tag="triu")
            ident_sb = pp.tile([P, P], F32, tag="ident")
            identb_sb = pp.tile([P, P], BF16, tag="identb")
            allones_sb = pp.tile([P, P], F32, tag="allones")
            iotaC_sb = pp.tile([P, C], F32, tag="iotaC")
            ones_sb = pp.tile([P, 1], F32, tag="ones")
            mask_sb = pp.tile([P, NT, E], F32, tag="mask")
            rank4_sb = pp.tile([P, NT, EPC], F32, tag="rank4")
            cw4_sb = pp.tile([P, NT, EPC], F32, tag="cw4")

            nc.sync.dma_start(out=x_sb, in_=x_d.rearrange("(n p) d -> p n d", p=P))
            nc.sync.dma_start(out=bias_sb, in_=bias_d)
            nc.sync.dma_start(out=triu_sb, in_=triu_d)
            nc.sync.dma_start(out=ident_sb, in_=ident_d)
            nc.sync.dma_start(out=identb_sb, in_=identb_d)
            nc.sync.dma_start(out=allones_sb, in_=allones_d)
            nc.sync.dma_start(out=iotaC_sb, in_=iotaC_d)
            nc.sync.dma_start(out=ones_sb, in_=ones_d)

            # ---------- early-phase tensors (released before expert loop) ----------
            early = tc.alloc_tile_pool(name="early", bufs=1)
            xT_sb = early.tile([P, ND, T], F32, tag="xT_sb")
            xTb_sb = early.tile([P, ND, T], BF16, tag="xTb_sb")
            gateT_sb = early.tile([P, ND, E], F32, tag="gateT")
            nc.sync.dma_start(out=xT_sb, in_=xT_d.rearrange("(n p) t -> p n t", p=P))
            nc.sync.dma_start(out=xTb_sb, in_=xTb_d.rearrange("(n p) t -> p n t", p=P))
            nc.sync.dma_start(out=gateT_sb, in_=gateT_d.rearrange("(n p) e -> p n e", p=P))

            # ---------- router ----------
            counts_ps = psC.tile([1, E], F32, tag="C")

            for tt in range(NT):
                lg_ps = psA.tile([P, E], F32, tag="A")
                for dc in range(ND):
                    nc.tensor.matmul(
                        lg_ps,
                        xT_sb[:, dc, tt * P:(tt + 1) * P],
                        gateT_sb[:, dc, :],
                        start=(dc == 0),
                        stop=(dc == ND - 1),
                    )
                logits = rt.tile([P, E], F32, tag="logits")
                nc.vector.tensor_tensor(logits, lg_ps, bias_sb, op=AL.add)
                max8 = rt.tile([P, 8], F32, tag="max8")
                nc.vector.max(out=max8, in_=logits)
                negm = rt.tile([P, 1], F32, tag="negm")
                nc.vector.tensor_scalar_mul(negm, max8[:, 0:1], -1.0)
                msk = mask_sb[:, tt, :]
                nc.vector.tensor_scalar(
                    msk, logits, max8[:, 3:4], None, op0=AL.is_ge
                )
                expv = rt.tile([P, E], F32, tag="expv")
                nc.scalar.activation(expv, logits, AF.Exp, bias=negm)
                mexp = rt.tile([P, E], F32, tag="mexp")
                nc.vector.tensor_mul(mexp, expv, msk)
                zs = rt.tile([P, 1], F32, tag="zs")
                nc.vector.reduce_sum(zs, mexp, axis=AX.X)
                rz = rt.tile([P, 1], F32, tag="rz")
                nc.vector.reciprocal(rz, zs)
                nc.vector.tensor_scalar(
                    cw4_sb[:, tt, :], mexp[:, 0:EPC], rz, None, op0=AL.mult
                )

                # counts (accumulated over tiles)
                nc.tensor.matmul(
                    counts_ps, ones_sb, msk, start=(tt == 0), stop=(tt == NT - 1)
                )

                # exclusive prefix ranks for this core's 4 experts (cols 0..3):
                # sum of full earlier tiles (all-ones lhsT) + strict within-tile
                # prefix (triu lhsT), all in one PSUM accumulation group.
                pre_ps = psA.tile([P, EPC], F32, tag="A")
                for i in range(tt):
                    nc.tensor.matmul(
                        pre_ps, allones_sb, mask_sb[:, i, 0:EPC],
                        start=(i == 0), stop=False,
                    )
                nc.tensor.matmul(
                    pre_ps, triu_sb, msk[:, 0:EPC], start=(tt == 0), stop=True
                )
                r4 = rt.tile([P, EPC], F32, tag="r4")
                nc.vector.tensor_scalar_add(r4, pre_ps, 1.0)
                nc.vector.tensor_mul(r4, r4, msk[:, 0:EPC])
                nc.vector.tensor_scalar_add(rank4_sb[:, tt, :], r4, -1.0)

            # ---------- counts stats ----------
            counts_sb = pp.tile([1, E], F32, tag="counts_sb")
            nc.vector.tensor_copy(counts_sb, counts_ps)
            nc.sync.dma_start(out=counts_d, in_=counts_sb)
            mean = rt.tile([1, 1], F32, tag="mean")
            nc.vector.reduce_sum(mean, counts_sb, axis=AX.X)
            nc.vector.tensor_scalar_mul(mean, mean, 1.0 / E)
            dev = rt.tile([1, E], F32, tag="dev")
            nc.vector.tensor_scalar(dev, counts_sb, mean, None, op0=AL.subtract)
            nc.vector.tensor_mul(dev, dev, dev)
            var = rt.tile([1, 1], F32, tag="var")
            nc.vector.reduce_sum(var, dev, axis=AX.X)
            nc.vector.tensor_scalar_mul(var, var, 1.0 / E)
            std = rt.tile([1, 1], F32, tag="std")
            nc.scalar.activation(std, var, AF.Sqrt)
            meps = rt.tile([1, 1], F32, tag="meps")
            nc.vector.tensor_scalar_add(meps, mean, 1e-6)
            rme = rt.tile([1, 1], F32, tag="rme")
            nc.vector.reciprocal(rme, meps)
            lb = rt.tile([1, 1], F32, tag="lb")
            nc.vector.tensor_mul(lb, std, rme)
            nc.sync.dma_start(out=lb_d, in_=lb)

            # ---------- shared expert (F-slice), writes acc ----------
            with tc.tile_pool(name="shared", bufs=1) as sh:
                wsgT_sb = sh.tile([P, ND, FS], BF16, tag="wsgT")
                wsuT_sb = sh.tile([P, ND, FS], BF16, tag="wsuT")
                wsdT_sb = sh.tile([P, FS // P, D], BF16, tag="wsdT")
                hsT = sh.tile([P, FS // P, T], BF16, tag="hsT")
                nc.sync.dma_start(
                    out=wsgT_sb, in_=wsgT_d.rearrange("(n p) f -> p n f", p=P)
                )
                nc.sync.dma_start(
                    out=wsuT_sb, in_=wsuT_d.rearrange("(n p) f -> p n f", p=P)
                )
                nc.sync.dma_start(
                    out=wsdT_sb, in_=wsdT_d.rearrange("(n p) d -> p n d", p=P)
                )
                for fs in range(FS // P):
                    for th in range(2):
                        tsl = slice(th * 512, (th + 1) * 512)
                        psg = psB.tile([P, 512], F32, tag="B")
                        psu = psB.tile([P, 512], F32, tag="B")
                        for dc in range(ND):
                            _mm(nc, psg, wsgT_sb[:, dc, fs * P:(fs + 1) * P],
                                xTb_sb[:, dc, tsl], dc == 0, dc == ND - 1)
                        for dc in range(ND):
                            _mm(nc, psu, wsuT_sb[:, dc, fs * P:(fs + 1) * P],
                                xTb_sb[:, dc, tsl], dc == 0, dc == ND - 1)
                        sil = rt.tile([P, 512], F32, tag="sil")
                        nc.scalar.activation(sil, psg, AF.Sigmoid)
                        nc.vector.tensor_mul(sil, sil, psg)
                        nc.vector.tensor_mul(hsT[:, fs, tsl], sil, psu)
                for tt in range(NT):
                    for dh in range(2):
                        dsl = slice(dh * 512, (dh + 1) * 512)
                        pso = psB.tile([P, 512], F32, tag="B")
                        for fs in range(FS // P):
                            _mm(nc, pso, hsT[:, fs, tt * P:(tt + 1) * P],
                                wsdT_sb[:, fs, dsl], fs == 0, fs == FS // P - 1)
                        nc.vector.tensor_copy(acc[:, tt, dsl], pso)

            early.release()

            # ---------- routed experts ----------
            # bufs=2 on the per-expert tensors lets expert e+1's gather/build
            # phases overlap expert e's compute tail (PE was ~25% idle with
            # single-buffered expert state).
            with tc.tile_pool(name="exp", bufs=2) as ep, \
                 tc.tile_pool(name="wstream", bufs=2) as ws, \
                 tc.tile_pool(name="wdstream", bufs=3) as wds:
                for e in range(EPC):
                    # one-hot gather matrix GT[t, s] = (rank[t] == s)
                    GT = ep.tile([P, NT, C], BF16, tag="GT")
                    for tt in range(NT):
                        nc.vector.tensor_scalar(
                            GT[:, tt, :], iotaC_sb, rank4_sb[:, tt, e:e + 1],
                            None, op0=AL.is_equal,
                        )
                    # weighted scatter matrix S_w[s, t] = GT[t, s] * cw[t],
                    # built as per-partition-scalar multiply then PE-transposed
                    sw = ep.tile([P, NC_CHUNK, T], BF16, tag="sw")
                    for tt in range(NT):
                        swT_t = rt.tile([P, C], BF16, tag="swT_t")
                        nc.vector.tensor_scalar(
                            swT_t, GT[:, tt, :], cw4_sb[:, tt, e:e + 1],
                            None, op0=AL.mult,
                        )
                        for cc in range(NC_CHUNK):
                            tr_ps = psA.tile([P, P], BF16, tag="A")
                            nc.tensor.transpose(
                                tr_ps, swT_t[:, cc * P:(cc + 1) * P], identb_sb
                            )
                            nc.vector.tensor_copy(
                                sw[:, cc, tt * P:(tt + 1) * P], tr_ps
                            )

                    # gathered+transposed tokens xgT[D, C]
                    xgT = ep.tile([P, ND, C], BF16, tag="xgT")
                    for dc in range(ND):
                        g_ps = psA.tile([P, C], F32, tag="A")
                        for tt in range(NT):
                            _mm(nc, g_ps, x_sb[:, tt, dc * P:(dc + 1) * P],
                                GT[:, tt, :], tt == 0, tt == NT - 1)
                        nc.vector.tensor_copy(xgT[:, dc, :], g_ps)

                    # swiglu hidden + down-proj, y accumulated in 4 PSUM banks
                    y_ps = [
                        [psB.tile([P, 512], F32, tag="B", name=f"y_ps_{cc}_{dh}")
                         for dh in range(2)]
                        for cc in range(NC_CHUNK)
                    ]
                    for fg in range(NFG):
                        wg_t = ws.tile([P, ND, FG], BF16, tag="wg")
                        wu_t = ws.tile([P, ND, FG], BF16, tag="wu")
                        fsl = slice(fg * FG, (fg + 1) * FG)
                        nc.sync.dma_start(
                            out=wg_t,
                            in_=wgT_d[e, :, fsl].rearrange("(n p) f -> p n f", p=P),
                        )
                        nc.sync.dma_start(
                            out=wu_t,
                            in_=wuT_d[e, :, fsl].rearrange("(n p) f -> p n f", p=P),
                        )
                        for fi in range(FG // P):
                            ft = fg * (FG // P) + fi
                            psg = psA.tile([P, C], F32, tag="A")
                            psu = psA.tile([P, C], F32, tag="A")
                            for dc in range(ND):
                                _mm(nc, psg, wg_t[:, dc, fi * P:(fi + 1) * P],
                                    xgT[:, dc, :], dc == 0, dc == ND - 1)
                            for dc in range(ND):
                                _mm(nc, psu, wu_t[:, dc, fi * P:(fi + 1) * P],
                                    xgT[:, dc, :], dc == 0, dc == ND - 1)
                            sil = rt.tile([P, C], F32, tag="silh")
                            nc.scalar.activation(sil, psg, AF.Sigmoid)
                            nc.vector.tensor_mul(sil, sil, psg)
                            hT = ep.tile([P, C], BF16, tag="hT")
                            nc.vector.tensor_mul(hT, sil, psu)
                            wd_t = wds.tile([P, D], BF16, tag="wd")
                            nc.sync.dma_start(
                                out=wd_t, in_=wdT_d[e, ft * P:(ft + 1) * P, :]
                            )
                            for cc in range(NC_CHUNK):
                                for dh in range(2):
                                    _mm(nc, y_ps[cc][dh],
                                        hT[:, cc * P:(cc + 1) * P],
                                        wd_t[:, dh * 512:(dh + 1) * 512],
                                        ft == 0, ft == NF - 1)
                    y_sb = ep.tile([P, NC_CHUNK, D], BF16, tag="y_sb")
                    for cc in range(NC_CHUNK):
                        for dh in range(2):
                            nc.vector.tensor_copy(
                                y_sb[:, cc, dh * 512:(dh + 1) * 512], y_ps[cc][dh]
                            )

                    for tt in range(NT):
                        for dh in range(2):
                            dsl = slice(dh * 512, (dh + 1) * 512)
                            pso = psB.tile([P, 512], F32, tag="B")
                            for cc in range(NC_CHUNK):
                                _mm(nc, pso, sw[:, cc, tt * P:(tt + 1) * P],
                                    y_sb[:, cc, dsl], cc == 0, cc == NC_CHUNK - 1)
                            nc.vector.tensor_add(acc[:, tt, dsl], acc[:, tt, dsl], pso)

            nc.sync.dma_start(out=out_d.rearrange("(n p) d -> p n d", p=P), in_=acc)

    nc.compile()
    return nc


def make_in_maps(inputs):
    x = np.ascontiguousarray(inputs["x"].reshape(T, D).astype(np.float32))
    gate_w = np.asarray(inputs["gate_w"], np.float32)
    expert_bias = np.asarray(inputs["expert_bias"], np.float32)
    wg = np.asarray(inputs["wg"], np.float32)
    wu = np.asarray(inputs["wu"], np.float32)
    wd = np.asarray(inputs["wd"], np.float32)
    ws_g = np.asarray(inputs["ws_g"], np.float32)
    ws_u = np.asarray(inputs["ws_u"], np.float32)
    ws_d = np.asarray(inputs["ws_d"], np.float32)

    xT = np.ascontiguousarray(x.T)
    triu = np.triu(np.ones((P, P), np.float32), k=1)
    ident = np.eye(P, dtype=np.float32)
    allones = np.ones((P, P), np.float32)
    iotaC = np.broadcast_to(np.arange(C, dtype=np.float32), (P, C)).copy()
    ones = np.ones((P, 1), np.float32)

    import ml_dtypes
    bf16 = ml_dtypes.bfloat16
    x_b = x.astype(bf16)
    xTb = np.ascontiguousarray(x.T).astype(bf16)
    in_maps = []
    for c in range(NCORES):
        own = list(range(c * EPC, (c + 1) * EPC))
        perm = own + [i for i in range(E) if i not in own]
        fsl = slice(c * FS, (c + 1) * FS)
        in_maps.append({
            "x": x_b,
            "xT": xT,
            "xTb": xTb,
            "gateT": np.ascontiguousarray(gate_w[perm].T),
            "bias_row": np.broadcast_to(expert_bias[perm], (P, E)).copy(),
            "wgT": np.ascontiguousarray(wg[own].transpose(0, 2, 1)).astype(bf16),
            "wuT": np.ascontiguousarray(wu[own].transpose(0, 2, 1)).astype(bf16),
            "wdT": np.ascontiguousarray(wd[own].transpose(0, 2, 1)).astype(bf16),
            "wsgT": np.ascontiguousarray(ws_g[fsl, :].T).astype(bf16),
            "wsuT": np.ascontiguousarray(ws_u[fsl, :].T).astype(bf16),
            "wsdT": np.ascontiguousarray(ws_d[:, fsl].T).astype(bf16),
            "triu": triu,
            "ident": ident,
            "identb": ident.astype(bf16),
            "allones": allones,
            "iotaC": iotaC,
            "ones": ones,
        })
    return in_maps


def kernel(x, gate_w, expert_bias, wg, wu, wd, ws_g, ws_u, ws_d, **run_kwargs):
    global _NC_CACHE
    inputs = dict(x=x, gate_w=gate_w, expert_bias=expert_bias, wg=wg, wu=wu,
                  wd=wd, ws_g=ws_g, ws_u=ws_u, ws_d=ws_d)
    in_maps = make_in_maps(inputs)
    if _NC_CACHE is None:
        _NC_CACHE = build_nc()
    nc = _NC_CACHE
    res = run_bass_kernel_spmd(nc, in_maps, list(range(NCORES)), **run_kwargs)
    parts = res.results
    out = np.zeros((T, D), np.float64)
    for c in range(NCORES):
        out += parts[c]["out_partial"].astype(np.float64)
    out = out.astype(np.float32).reshape(B, S, D)
    counts = parts[0]["counts"].reshape(E).astype(np.float32)
    lb = np.float32(parts[0]["lb"].reshape(-1)[0])
    if run_kwargs:
        return (out, counts, lb), res
    return out, counts, lb
